# revision 30
# baseline (speedup 1.0000x reference)
"""Deformable attention kernel for Trainium2 (Bass/Tile), 8 NeuronCores.

Sharding: data-parallel over batch (bz=8 -> one batch per core), no
collectives. Per core:
  1. QW = query @ [Wo|Wa]^T + [bo|ba]      (PE, true fp32 -- position precision)
  2. coeff pipeline (positions, floor, interp weights, softmax)   (DVE/ACT)
  3. vtable = value @ Wv^T + bv  -> per-head row tables in DRAM   (PE f32r)
  4. idx relayout via identity-slice selector matmuls             (PE fp32)
  5. per-head pair-gather (dma_gather, 512B descs, elem_step=64)  (SWDGE)
  6. weighted combine (broadcast-AP tensor_tensor ops)            (DVE)
  7. transpose combined via PE, out = combined @ Wout^T + bout    (PE bf16)
"""

import os
import numpy as np

E = 1024
H = 16
D = 64
K = 4
LQ = 2048
LV = 4096
BZ = 8
NCORES = 8
QT = LQ // 128          # 16 q-tiles
ECH = E // 128          # 8 e-chunks
LT = LV // 128          # 32 Lv-tiles
RND = 12582912.0        # 1.5 * 2^23, round-to-nearest trick

_CACHE = {}


def _build_program():
    import concourse.bass as bass
    import concourse.mybir as mybir
    import concourse.tile as tile
    from concourse import bacc
    from concourse.masks import make_identity
    from concourse.tile import add_dep_helper
    from contextlib import ExitStack

    dt = mybir.dt
    f32 = dt.float32
    f32r = dt.float32r
    i16 = dt.int16
    bf16 = dt.bfloat16
    Alu = mybir.AluOpType
    Act = mybir.ActivationFunctionType

    lvl = int(os.environ.get("KLVL", "9"))

    nc = bacc.Bacc()

    # ---- I/O ----
    qT_d = nc.dram_tensor("qT", [E, LQ], f32, kind="ExternalInput")
    vT_d = nc.dram_tensor("vT", [E, LV], f32r, kind="ExternalInput")
    refp_d = nc.dram_tensor("refp", [128, QT], f32, kind="ExternalInput")
    snip_d = nc.dram_tensor("snip", [1, 1], f32, kind="ExternalInput")
    wvT_d = nc.dram_tensor("wvT", [E, E], f32r, kind="ExternalInput")
    woaT_d = nc.dram_tensor("woaT", [E, 128], f32, kind="ExternalInput")
    woutT_d = nc.dram_tensor("woutT", [E, E], bf16, kind="ExternalInput")
    bv_d = nc.dram_tensor("bv", [1, E], f32r, kind="ExternalInput")
    boba_d = nc.dram_tensor("boba", [1, 128], f32, kind="ExternalInput")
    bout_d = nc.dram_tensor("bout", [1, E], bf16, kind="ExternalInput")
    onesr_d = nc.dram_tensor("onesr", [1, 128], f32r, kind="ExternalInput")
    onesb_d = nc.dram_tensor("onesb", [1, 128], bf16, kind="ExternalInput")
    out_d = nc.dram_tensor("out", [LQ, E], f32, kind="ExternalOutput")
    # per-head row table, one zero row at top and bottom: row r <-> v[r-1]
    vtab_d = nc.dram_tensor("vtab", [H, LV + 2, D], f32, kind="Internal")

    with ExitStack() as ctx:
        tc = ctx.enter_context(tile.TileContext(nc))

        const = ctx.enter_context(tc.tile_pool(name="const", bufs=1))
        mid = ctx.enter_context(tc.tile_pool(name="mid", bufs=1))
        combp = ctx.enter_context(tc.tile_pool(name="comb", bufs=1))
        dbgp = ctx.enter_context(tc.tile_pool(name="dbg", bufs=1))

        # ---------- constants ----------
        ident = const.tile([128, 128], f32)
        make_identity(nc, ident[:])
        identb = const.tile([128, 128], bf16)
        make_identity(nc, identb[:])
        ones_row = const.tile([1, 128], f32)
        nc.vector.memset(ones_row[:], 1.0)

        wout_sb = const.tile([128, ECH, E], bf16)
        nc.sync.dma_start(
            wout_sb[:], woutT_d[:].rearrange("(c p) n -> p c n", p=128))

        bvr = const.tile([1, E], f32r)
        nc.sync.dma_start(bvr[:], bv_d[:])
        bobar = const.tile([1, 128], f32)
        nc.sync.dma_start(bobar[:], boba_d[:])
        boutr = const.tile([1, E], bf16)
        nc.sync.dma_start(boutr[:], bout_d[:])
        onesr = const.tile([1, 128], f32r)
        nc.sync.dma_start(onesr[:], onesr_d[:])
        onesb = const.tile([1, 128], bf16)
        nc.sync.dma_start(onesb[:], onesb_d[:])

        snipt = const.tile([1, 1], f32)
        nc.sync.dma_start(snipt[:], snip_d[:])
        srec = const.tile([1, 1], f32)
        nc.vector.reciprocal(srec[:], snipt[:])
        sc4095 = const.tile([128, 1], f32)
        with tc.tile_pool(name="bcps", bufs=1, space="PSUM") as bcps:
            bps = bcps.tile([128, 1], f32)
            nc.tensor.matmul(bps[:], ones_row[:], srec[:], start=True, stop=True)
            nc.vector.tensor_scalar(sc4095[:], bps[:], 4095.0, None, Alu.mult)

        # selector matrices, replicated across the 8 Q7-core partition groups:
        # selr[pg][p, r*16+c] = 1 iff p == pg*16 + c
        selr = []
        for pg in range(8):
            s = const.tile([128, 128], f32, tag=f"selr{pg}")
            nc.vector.tensor_copy(
                s[:].rearrange("p (r c) -> p r c", c=16),
                ident[:, pg * 16:(pg + 1) * 16].unsqueeze(1).to_broadcast(
                    [128, 8, 16]),
            )
            selr.append(s)

        woa_sb = const.tile([128, ECH, 128], f32)
        nc.sync.dma_start(woa_sb[:], woaT_d[:].rearrange("(c p) n -> p c n", p=128))

        # persistent mid tensors
        cpair = mid.tile([128, QT, H * K, 2], f32)   # interp coeffs c0/c1
        idx_t = mid.tile([128, H * 512], i16)        # gather indices

        # ---------- phase B: QW = query @ [Wo|Wa]^T + [bo|ba]  (true fp32) ----------
        qwpool = ctx.enter_context(tc.tile_pool(name="qw", bufs=1))
        qw_sb = qwpool.tile([128, QT, 128], f32)
        with tc.tile_pool(name="qtiles", bufs=3) as qtp, \
             tc.tile_pool(name="qwps", bufs=2, space="PSUM") as qwps:
            for qt in range(QT):
                qtile = qtp.tile([128, ECH, 128], f32, tag="qtile")
                nc.sync.dma_start(
                    qtile[:],
                    qT_d[:].rearrange("(c p) q -> p c q", p=128)[
                        :, :, qt * 128:(qt + 1) * 128],
                )
                pq = qwps.tile([128, 128], f32)
                for ch in range(ECH):
                    nc.tensor.matmul(
                        pq[:], qtile[:, ch, :], woa_sb[:, ch, :],
                        start=(ch == 0), stop=False)
                nc.tensor.matmul(
                    pq[:], ones_row[:], bobar[:], start=False, stop=True)
                if qt % 2 == 0:
                    nc.vector.tensor_copy(qw_sb[:, qt, :], pq[:])
                else:
                    nc.scalar.copy(qw_sb[:, qt, :], pq[:])
        if lvl == 1:
            nc.sync.dma_start(out_d[0:128, :], qw_sb[:, 0:8, :])

        # ---------- phase E: vtable = value @ Wv^T + bv  (f32r) ----------
        # half-outer loop: all of heads 0-7 are written first so their
        # gathers can start while heads 8-15 are still being projected.
        vwrites = [[], []]
        if lvl >= 2:
            with tc.tile_pool(name="wv", bufs=1) as wvp, \
                 tc.tile_pool(name="vtiles", bufs=3) as vtp, \
                 tc.tile_pool(name="vstage", bufs=3) as vsp, \
                 tc.tile_pool(name="vps", bufs=4, space="PSUM") as vps:
                wv_sb = wvp.tile([128, ECH, E], f32r)
                nc.sync.dma_start(
                    wv_sb[:], wvT_d[:].rearrange("(c p) n -> p c n", p=128))

                zrow = wvp.tile([16, D], f32)
                nc.vector.memset(zrow[:], 0.0)
                z0 = nc.sync.dma_start(vtab_d[:, 0, :], zrow[:])
                z1 = nc.sync.dma_start(vtab_d[:, LV + 1, :], zrow[:])
                vwrites[0] += [z0, z1]
                vwrites[1] += [z0, z1]

                vtab_lhd = vtab_d[:].rearrange("h l d -> l h d")
                half_outer = os.environ.get("KHALF", "0") == "1"

                def _mm_half(lt, half, vtile, stage):
                    pv = vps.tile([128, 512], f32, tag="vps")
                    for ch in range(ECH):
                        nc.tensor.matmul(
                            pv[:],
                            vtile[:, ch, :],
                            wv_sb[:, ch, half * 512:(half + 1) * 512],
                            start=(ch == 0), stop=False)
                    nc.tensor.matmul(
                        pv[:], onesr[:],
                        bvr[0:1, half * 512:(half + 1) * 512],
                        start=False, stop=True)
                    if half == 0:
                        nc.scalar.copy(stage[:, 0:512], pv[:])
                    else:
                        nc.vector.tensor_copy(stage[:, 512:1024], pv[:])

                def _write(lt, half, stage):
                    w = nc.sync.dma_start(
                        vtab_lhd[1 + lt * 128: 1 + lt * 128 + 128,
                                 half * 8:(half + 1) * 8, :],
                        stage[:, half * 512:(half + 1) * 512].rearrange(
                            "p (h d) -> p h d", d=D),
                    )
                    vwrites[half].append(w)

                def _load_vtile(lt):
                    vtile = vtp.tile([128, ECH, 128], f32r, tag="vtile")
                    nc.sync.dma_start(
                        vtile[:],
                        vT_d[:].rearrange("(c p) l -> p c l", p=128)[
                            :, :, lt * 128:(lt + 1) * 128],
                    )
                    return vtile

                if half_outer:
                    for half in range(2):
                        for lt in range(LT):
                            stage = vsp.tile([128, E], f32, tag="vstage")
                            _mm_half(lt, half, _load_vtile(lt), stage)
                            _write(lt, half, stage)
                else:
                    for lt in range(LT):
                        vtile = _load_vtile(lt)
                        stage = vsp.tile([128, E], f32, tag="vstage")
                        for half in range(2):
                            _mm_half(lt, half, vtile, stage)
                        # one merged write covering all 16 heads
                        w = nc.sync.dma_start(
                            vtab_lhd[1 + lt * 128: 1 + lt * 128 + 128, :, :],
                            stage[:].rearrange("p (h d) -> p h d", d=D),
                        )
                        vwrites[0].append(w)
                        vwrites[1].append(w)
        if lvl == 2:
            nc.sync.dma_start(out_d[0:128, :], qw_sb[:, 0:8, :])

        # ---------- phase C: coefficient pipeline ----------
        if lvl >= 3:
            with tc.tile_pool(name="coeff", bufs=1) as cfp:
                qwo = qw_sb[:, :, 0:64]
                lg = qw_sb[:, :, 64:128]

                refl = cfp.tile([128, QT], f32)
                nc.sync.dma_start(refl[:], refp_d[:])
                ref2 = cfp.tile([128, QT], f32)
                nc.vector.tensor_scalar(ref2[:], refl[:], 4095.0, None, Alu.mult)

                x = cfp.tile([128, QT, 64], f32)
                nc.vector.tensor_scalar(x[:], qwo, sc4095[:], None, Alu.mult)
                nc.vector.tensor_tensor(
                    x[:], x[:],
                    ref2[:].unsqueeze(-1).to_broadcast([128, QT, 64]), Alu.add)

                rz = cfp.tile([128, QT, 64], f32)
                nc.vector.tensor_scalar(rz[:], x[:], RND, None, Alu.add)
                nc.vector.tensor_scalar(rz[:], rz[:], RND, None, Alu.subtract)
                gt = cfp.tile([128, QT, 64], f32)
                nc.vector.tensor_tensor(gt[:], rz[:], x[:], Alu.is_gt)
                x0 = cfp.tile([128, QT, 64], f32)
                nc.vector.tensor_tensor(x0[:], rz[:], gt[:], Alu.subtract)
                w_ = cfp.tile([128, QT, 64], f32)
                nc.vector.tensor_tensor(w_[:], x[:], x0[:], Alu.subtract)

                oka = cfp.tile([128, QT, 64], f32)
                okb = cfp.tile([128, QT, 64], f32)
                ok0 = cfp.tile([128, QT, 64], f32)
                nc.vector.tensor_scalar(oka[:], x0[:], 0.0, None, Alu.is_ge)
                nc.vector.tensor_scalar(okb[:], x0[:], 4095.0, None, Alu.is_le)
                nc.vector.tensor_tensor(ok0[:], oka[:], okb[:], Alu.mult)
                ok1 = cfp.tile([128, QT, 64], f32)
                nc.vector.tensor_scalar(oka[:], x0[:], -1.0, None, Alu.is_ge)
                nc.vector.tensor_scalar(okb[:], x0[:], 4094.0, None, Alu.is_le)
                nc.vector.tensor_tensor(ok1[:], oka[:], okb[:], Alu.mult)

                rf = cfp.tile([128, QT, 64], f32)
                nc.vector.tensor_scalar(rf[:], x0[:], -1.0, 4095.0, Alu.max,
                                        Alu.min)
                nc.vector.tensor_scalar(rf[:], rf[:], 1.0, None, Alu.add)

                # softmax over K
                lg4 = lg.rearrange("p t (h k) -> p t h k", k=K)
                mx = cfp.tile([128, QT, H], f32)
                nc.vector.tensor_reduce(mx[:], lg4, mybir.AxisListType.X, Alu.max)
                es = cfp.tile([128, QT, 64], f32)
                es4 = es[:].rearrange("p t (h k) -> p t h k", k=K)
                nc.vector.tensor_tensor(
                    es4, lg4,
                    mx[:].unsqueeze(-1).to_broadcast([128, QT, H, K]),
                    Alu.subtract)
                nc.scalar.activation(es[:], es[:], Act.Exp)
                sm = cfp.tile([128, QT, H], f32)
                nc.vector.tensor_reduce(sm[:], es4, mybir.AxisListType.X, Alu.add)
                rs = cfp.tile([128, QT, H], f32)
                nc.vector.reciprocal(rs[:], sm[:])
                aw = cfp.tile([128, QT, 64], f32)
                nc.vector.tensor_tensor(
                    aw[:].rearrange("p t (h k) -> p t h k", k=K), es4,
                    rs[:].unsqueeze(-1).to_broadcast([128, QT, H, K]), Alu.mult)

                omw = cfp.tile([128, QT, 64], f32)
                nc.scalar.activation(omw[:], w_[:], Act.Copy, bias=1.0,
                                     scale=-1.0)

                tmp = cfp.tile([128, QT, 64], f32)
                nc.vector.tensor_tensor(tmp[:], aw[:], omw[:], Alu.mult)
                nc.vector.tensor_tensor(cpair[:, :, :, 0], tmp[:], ok0[:],
                                        Alu.mult)
                nc.vector.tensor_tensor(tmp[:], aw[:], w_[:], Alu.mult)
                nc.vector.tensor_tensor(cpair[:, :, :, 1], tmp[:], ok1[:],
                                        Alu.mult)

                if lvl == 3:
                    nc.sync.dma_start(
                        out_d[0:128, :],
                        cpair[:].rearrange("p a b c -> p (a b c)")[:, 0:1024])

                # ---------- phase D: idx relayout (selector matmuls) ----------
                # idx position for sample (q, k): partition q%16,
                # column h*512 + k*128 + (q//128)*8 + (q//16)%8
                if lvl >= 4:
                    rf_flat = rf[:].rearrange("p t j -> p (t j)")
                    idx5 = idx_t[:].rearrange(
                        "p (h k qh g) -> p qh h k g", h=H, k=K, qh=QT, g=8)
                    with tc.tile_pool(name="idxps", bufs=2, space="PSUM") as ixp:
                        for pg in range(8):
                            for half in range(2):
                                ps = ixp.tile([128, 512], f32, tag="idxps")
                                nc.tensor.matmul(
                                    ps[:], selr[pg][:],
                                    rf_flat[:, half * 512:(half + 1) * 512],
                                    start=True, stop=True)
                                nc.vector.tensor_copy(
                                    idx5[:, half * 8:(half + 1) * 8, :, :, pg],
                                    ps[:].rearrange("p (q h k) -> p q h k",
                                                    h=H, k=K),
                                )
                    if lvl == 4:
                        dbg = dbgp.tile([128, 1024], f32)
                        nc.vector.tensor_copy(dbg[:], idx_t[:, 0:1024])
                        nc.sync.dma_start(out_d[0:128, :], dbg[:])

        # ---------- phase F: gather + combine ----------
        if lvl >= 5:
            nogather = os.environ.get("KNOGATHER", "0") == "1"
            nocombine = os.environ.get("KNOCOMBINE", "0") == "1"
            with tc.tile_pool(name="gat", bufs=2) as gp, \
                 tc.tile_pool(name="work", bufs=1) as wkp:
                combined = combp.tile([128, QT, H, D], bf16)
                for h in range(H):
                    g = gp.tile([128, 64, 128], f32, tag="g")
                    if nogather:
                        nc.vector.memset(g[:], 0.125)
                    else:
                        src = vtab_d[h].copy()
                        src.ap[0] = (D, LV + 1)
                        src.ap[1] = (1, 2 * D)
                        gi = nc.gpsimd.dma_gather(
                            g[:], src, idx_t[:, h * 512:(h + 1) * 512],
                            num_idxs=512 * 16, num_idxs_reg=512 * 16,
                            elem_size=2 * D, elem_step=D,
                            single_packet=False)
                        for wr in vwrites[h // 8]:
                            add_dep_helper(gi.ins, wr.ins,
                                           reason="gather after vtab write")

                    if nocombine:
                        nc.vector.tensor_copy(combined[:, :, h, :],
                                              g[:, 0:QT, 0:D])
                        continue
                    # late heads: run the adds on the (by then idle) gpsimd
                    # engine to take pressure off the DVE critical path
                    adder = nc.gpsimd if h >= 13 else nc.vector
                    acc = wkp.tile([128, QT, 2, D], f32, tag="acc")
                    pk = wkp.tile([128, QT, 2, D], f32, tag="pk")
                    for k in range(K):
                        g4 = g[:, k * QT:(k + 1) * QT, :].rearrange(
                            "p s (nb d) -> p s nb d", nb=2)
                        cc = cpair[:, :, h * K + k, :].unsqueeze(-1).to_broadcast(
                            [128, QT, 2, D])
                        if k == 0:
                            nc.vector.tensor_tensor(acc[:], g4, cc, Alu.mult)
                        else:
                            nc.vector.tensor_tensor(pk[:], g4, cc, Alu.mult)
                            adder.tensor_tensor(acc[:], acc[:], pk[:], Alu.add)
                    adder.tensor_tensor(
                        combined[:, :, h, :], acc[:, :, 0, :], acc[:, :, 1, :],
                        Alu.add)
            if lvl == 5:
                nc.sync.dma_start(
                    out_d[0:128, :],
                    combined[:].rearrange("p a b c -> p (a b c)")[:, 0:1024])

        # ---------- phase G: out = combined @ Wout^T + bout ----------
        if lvl >= 9:
            with tc.tile_pool(name="piece", bufs=4) as pcp, \
                 tc.tile_pool(name="ostage", bufs=2) as osp, \
                 tc.tile_pool(name="tps", bufs=4, space="PSUM") as tps, \
                 tc.tile_pool(name="ops", bufs=2, space="PSUM") as ops:
                for qt in range(QT):
                    pcs = []
                    for ch in range(ECH):
                        tp = tps.tile([128, 128], bf16, tag="tp")
                        nc.tensor.transpose(
                            tp[:],
                            combined[:, qt, 2 * ch:2 * ch + 2, :].rearrange(
                                "p h d -> p (h d)"),
                            identb[:])
                        pc = pcp.tile([128, 128], bf16, tag="pc")
                        if ch % 2 == 0:
                            nc.vector.tensor_copy(pc[:], tp[:])
                        else:
                            nc.scalar.copy(pc[:], tp[:])
                        pcs.append(pc)
                    ost = osp.tile([128, E], f32, tag="ost")
                    for half in range(2):
                        po = ops.tile([128, 512], f32, tag="ops")
                        for ch in range(ECH):
                            nc.tensor.matmul(
                                po[:], pcs[ch][:],
                                wout_sb[:, ch, half * 512:(half + 1) * 512],
                                start=(ch == 0), stop=False)
                        nc.tensor.matmul(
                            po[:], onesb[:],
                            boutr[0:1, half * 512:(half + 1) * 512],
                            start=False, stop=True)
                        if half == 0:
                            nc.vector.tensor_copy(ost[:, 0:512], po[:])
                        else:
                            nc.scalar.copy(ost[:, 512:1024], po[:])
                    nc.sync.dma_start(out_d[qt * 128:(qt + 1) * 128, :], ost[:])

    nc.finalize()
    return nc


def _get_program():
    if "nc" not in _CACHE:
        _CACHE["nc"] = _build_program()
    return _CACHE["nc"]


def _make_in_maps(inputs):
    query = np.asarray(inputs["query"], np.float32)
    value = np.asarray(inputs["value"], np.float32)
    refp = np.asarray(inputs["reference_point"], np.float32)
    snip = np.asarray(inputs["snippet_num"], np.float32)
    Wv = np.asarray(inputs["Wv"], np.float32)
    bv = np.asarray(inputs["bv"], np.float32)
    Wo = np.asarray(inputs["Wo"], np.float32)
    bo = np.asarray(inputs["bo"], np.float32)
    Wa = np.asarray(inputs["Wa"], np.float32)
    ba = np.asarray(inputs["ba"], np.float32)
    Wout = np.asarray(inputs["Wout"], np.float32)
    bout = np.asarray(inputs["bout"], np.float32)

    import ml_dtypes
    bf = ml_dtypes.bfloat16

    wvT = np.ascontiguousarray(Wv.T)
    woaT = np.ascontiguousarray(np.concatenate([Wo, Wa], axis=0).T)
    woutT = np.ascontiguousarray(Wout.T).astype(bf)
    bv_r = np.ascontiguousarray(bv[None, :])
    boba_r = np.ascontiguousarray(np.concatenate([bo, ba])[None, :])
    bout_r = np.ascontiguousarray(bout[None, :]).astype(bf)
    ones_r = np.ones((1, 128), np.float32)
    ones_b = np.ones((1, 128), bf)

    in_maps = []
    for c in range(NCORES):
        in_maps.append({
            "qT": np.ascontiguousarray(query[:, c, :].T),
            "vT": np.ascontiguousarray(value[:, c, :].T),
            "refp": np.ascontiguousarray(refp[c, :, 0].reshape(QT, 128).T),
            "snip": np.ascontiguousarray(snip[c].reshape(1, 1)),
            "wvT": wvT,
            "woaT": woaT,
            "woutT": woutT,
            "bv": bv_r,
            "boba": boba_r,
            "bout": bout_r,
            "onesr": ones_r,
            "onesb": ones_b,
        })
    return in_maps


def kernel(**inputs) -> np.ndarray:
    from concourse.bass_utils import run_bass_kernel_spmd

    nc = _get_program()
    in_maps = _make_in_maps(inputs)
    res = run_bass_kernel_spmd(nc, in_maps, core_ids=list(range(NCORES)))
    out = np.stack([r["out"] for r in res.results], axis=1)
    return np.ascontiguousarray(out.astype(np.float32))
